# revision 1
# baseline (speedup 1.0000x reference)
"""MoE (top-2 of 8 experts) Trainium2 kernel.

Strategy: expert-parallel across the 8 NeuronCores. The router
(softmax + top-2 over [T, 8] logits) is metadata computed on host to
build the dispatch; core e receives only the tokens routed to expert e
(gathered, transposed, zero-padded to a common capacity C) plus that
expert's weights, pre-transposed so the device does no transposes:

  core e inputs:  xT  [H, C]   = x[idx_e].T        (padded)
                  w1T [H, I]   = w1[e].T
                  w2T [I, H]   = w2[e].T
                  gates [1, C]  renormalized top-2 weight per token
  core e output:  yT  [H, C]  = (gate * (silu(x_e @ w1[e].T) @ w2[e].T)).T

On device (per core, fp32 storage, float32r matmuls, only the exact
even-rounded token count is computed — no padding columns):
  stage 1: hT[i_tile, c_chunk] = silu(w1T.T @ xT)    (I on partitions)
  stage 2: yT[h_tile, c_chunk] = w2T.T @ hT, times the per-token gate
           (w2 stationary, hT moving: the stream covers the ragged token
           dim; gate is broadcast to all partitions by a 0-stride DMA)

The host transposes and scatter-adds the two expert contributions per
token.
"""

import numpy as np

import concourse.mybir as mybir
from concourse import bacc
from concourse.tile import TileContext
from concourse.bass_utils import run_bass_kernel_spmd

T, H, I, E = 4096, 1024, 1408, 8
TOPK = 2
P = 128
CHUNK = 512
N_CORES = 8
F32 = mybir.dt.float32
F32R = mybir.dt.float32r
AF = mybir.ActivationFunctionType

# most recently built device program (for test harnesses / cost-model timing)
LAST_NC = None


def _chunk_sizes(C):
    """Split C into ceil(C/512) chunks, multiples of 128, as even as
    possible. Balanced chunks keep every stage-1 matmul's moving dim >=256
    (the fp32r full-rate threshold) instead of a slow ragged tail."""
    n = -(-C // CHUNK)
    base = (C // n) // P * P
    rem = (C - n * base) // P
    return [base + P if j < rem else base for j in range(n)]


def _count_chunks(count):
    """Chunk an exact token count (no alignment needed: both stages
    stream the token dim). Front chunks are full 512 so stage-1 groups outlast
    the w1 tile arrival period (no DMA-pacing stalls during the weight
    stream); the tail is split to keep every chunk >=256 (the fp32r
    full-rate threshold) whenever count allows."""
    full, rem = divmod(count, CHUNK)
    if rem == 0:
        return [CHUNK] * full
    if rem >= 256 or full == 0:
        return [CHUNK] * full + [rem]
    # rem < 256: borrow from one full chunk so both tail chunks stay >=256
    return [CHUNK] * (full - 1) + [256 + rem, 256]


def build_moe_expert_kernel(count, h=H, i_dim=I):
    """One-expert MLP over `count` gathered tokens (any positive int —
    DRAM buffers are padded to a 128 multiple, but only `count` columns
    are computed). h, i_dim overridable for small-scale simulation tests;
    both must be multiples of 128. count must be even (fp32r matmuls
    reject odd free/partition sizes)."""
    C = -(-count // P) * P  # DRAM/layout capacity
    assert count % 2 == 0 and h % P == 0 and i_dim % P == 0
    HK = h // P
    IT = i_dim // P

    nc = bacc.Bacc("TRN2", target_bir_lowering=False, debug=False, num_devices=N_CORES)
    # Matmul inputs are stored as float32r (same 32-bit layout; the PE
    # rounds to its reduced internal precision). Typing the whole producer
    # chain as f32r satisfies the BIR verifier's rounding check.
    xT = nc.dram_tensor("xT", [h, C], F32R, kind="ExternalInput").ap()
    w1T = nc.dram_tensor("w1T", [h, i_dim], F32R, kind="ExternalInput").ap()
    w2T = nc.dram_tensor("w2T", [i_dim, h], F32R, kind="ExternalInput").ap()
    gates = nc.dram_tensor("gates", [1, C], F32, kind="ExternalInput").ap()
    # host-packed first-group operands: per partition p (= h row p),
    # [w1T[p, 0:128] | xT[p, 0:cs0]] — one DMA arms the first matmul
    cs0_pre = _count_chunks(count)[0]
    prelude = nc.dram_tensor("prelude", [P, P + cs0_pre], F32R, kind="ExternalInput").ap()
    # output is yT [h, C]: stage 2 streams over the ragged token dim, so
    # tokens land on the free axis (the host transposes back)
    yT = nc.dram_tensor("yT", [h, C], F32, kind="ExternalOutput").ap()

    xT_v = xT.rearrange("(ho p) c -> p ho c", p=P)  # [128, HK, C]
    w1T_v = w1T.rearrange("(ho p) i -> p ho i", p=P)  # [128, HK, I]
    w2T_v = w2T.rearrange("(io p) h -> p io h", p=P)  # [128, IT, H]
    yT_v = yT.rearrange("(ho p) c -> ho p c", p=P)  # [HK, 128, C]

    h_chunks = _chunk_sizes(h)  # h-chunks for stage 2 output
    c_chunks = _count_chunks(count)
    max_cs = max(c_chunks)
    c_starts = [sum(c_chunks[:j]) for j in range(len(c_chunks))]
    # per-partition SBUF bytes: weights + broadcast gates + h/sg bufs; give
    # the x and y pools extra bufs only while the 192 KB budget holds
    base = 4 * (HK * i_dim + IT * h + C + 2 * IT * max_cs + 2 * CHUNK)
    x_bufs = 3 if base + 3 * 4 * HK * max_cs + 2 * 4 * CHUNK < 190 * 1024 else 2
    fixed = base + x_bufs * 4 * HK * max_cs
    y_bufs = 4 if fixed + 4 * 4 * CHUNK < 190 * 1024 else 2
    with TileContext(nc) as tc:
        with (
            tc.tile_pool(name="wpool", bufs=1) as wpool,
            tc.tile_pool(name="xpool", bufs=x_bufs) as xpool,
            tc.tile_pool(name="hpool", bufs=2) as hpool,
            tc.tile_pool(name="ypool", bufs=y_bufs) as ypool,
            tc.tile_pool(name="sgpool", bufs=2) as sgpool,
            tc.tile_pool(name="ps1", bufs=4, space="PSUM") as ps1pool,
            tc.tile_pool(name="ps2", bufs=4, space="PSUM") as ps2pool,
        ):
            # per-token gate replicated to all partitions: one DMA reading
            # the same DRAM row 128x (0-stride partition source)
            gb = wpool.tile([P, C], F32)
            w1s = wpool.tile([P, HK, i_dim], F32R)
            w2s = wpool.tile([P, IT, h], F32R)
            xs_tiles = {}

            def load_x(ci, split=True):
                # per-hk DMAs deliver the chunk incrementally so stage-1
                # groups can start before the whole chunk lands
                xs = xpool.tile([P, HK, max_cs], F32R, tag="xs", name=f"xs{ci}")
                cs, c0 = c_chunks[ci], c_starts[ci]
                if split:
                    for hk in range(HK):
                        nc.sync.dma_start(xs[:, hk, :cs], xT_v[:, hk, c0 : c0 + cs])
                else:
                    nc.sync.dma_start(xs[:, :, :cs], xT_v[:, :, c0 : c0 + cs])
                xs_tiles[ci] = xs

            def load_w1(it):
                nc.sync.dma_start(
                    w1s[:, :, it * P : (it + 1) * P],
                    w1T_v[:, :, it * P : (it + 1) * P],
                )

            # DMA issue order = consumption order. Interleave chunk-0 x
            # slices with the leading w1 i-tiles so the first stage-1
            # accumulation group starts after ~0.7 MB instead of ~6 MB;
            # then the rest of w1, the remaining x chunks, then w2 (per
            # h-half, consumed by stage 2).
            xs0 = xpool.tile([P, HK, max_cs], F32R, tag="xs", name="xs0")
            cs0 = c_chunks[0]
            pre = wpool.tile([P, P + cs0], F32R)
            nc.sync.dma_start(pre[:], prelude[:])
            # w1 it0's hk0 slice lives in the prelude; load only hk1..
            nc.sync.dma_start(w1s[:, 1:, 0:P], w1T_v[:, 1:, 0:P])
            for hk in range(1, HK):
                nc.sync.dma_start(xs0[:, hk, :cs0], xT_v[:, hk, 0:cs0])
                if hk == min(2, HK - 1) and IT > 1:
                    load_w1(1)
            xs_tiles[0] = xs0
            for it in range(2, IT):
                load_w1(it)
            # w2 per h-half per i-tile: stage 2 consumes one h-chunk across
            # i-tiles in order, so fine-grained delivery unblocks each
            # accumulation group as early as possible
            h_starts = [sum(h_chunks[:j]) for j in range(len(h_chunks))]
            # only as many x chunks up front as there are pool slots — a
            # queued DMA waiting on a busy slot would head-of-line block
            # the w2 stream behind it; later chunks prefetch inside stage 1
            for ci in range(1, min(x_bufs, len(c_chunks))):
                load_x(ci)
            # broadcast-gate load sits after the stage-1 streams (it is
            # only needed when the first stage-2 group's psum is evacuated)
            nc.sync.dma_start(gb[:], gates[0].partition_broadcast(P))
            for h0, hcs in zip(h_starts, h_chunks):
                for it in range(IT):
                    nc.sync.dma_start(
                        w2s[:, it, h0 : h0 + hcs], w2T_v[:, it, h0 : h0 + hcs]
                    )

            hs_tiles = {}

            def stage1(ci):
                nxt = ci + 1
                if nxt < len(c_chunks) and nxt not in xs_tiles:
                    load_x(nxt)
                cs = c_chunks[ci]
                xs = xs_tiles[ci]
                # hT = silu(w1T.T @ xT)  -> [I, cs], I on partitions
                hs = hpool.tile([P, IT, max_cs], F32R, tag="hs", name=f"hs{ci}")
                for it in range(IT):
                    ps1 = ps1pool.tile([P, CHUNK], F32, tag="ps1")
                    for hk in range(HK):
                        # (it0, hk0) weights and chunk-0's hk0 x-slice live
                        # in the prelude tile (w1s[:, 0, 0:P] is never DMA'd)
                        if hk == 0 and it == 0:
                            lhsT = pre[:, 0:P]
                        else:
                            lhsT = w1s[:, hk, it * P : (it + 1) * P]
                        if ci == 0 and hk == 0:
                            rhs = pre[:, P : P + cs]
                        else:
                            rhs = xs[:, hk, :cs]
                        nc.tensor.matmul(
                            ps1[:, :cs],
                            lhsT,
                            rhs,
                            start=(hk == 0),
                            stop=(hk == HK - 1),
                        )
                    # silu(z) = z * sigmoid(z); CoreSim has no Silu table,
                    # so build it from Sigmoid (ACT) + multiply (DVE)
                    sg = sgpool.tile([P, CHUNK], F32, tag="sg")
                    nc.scalar.activation(sg[:, :cs], ps1[:, :cs], AF.Sigmoid)
                    nc.vector.tensor_mul(
                        out=hs[:, it, :cs], in0=ps1[:, :cs], in1=sg[:, :cs]
                    )
                hs_tiles[ci] = hs

            def stage2(ci):
                # yT = (w2T.T @ hT) * gate -> [H, cs], h on partitions.
                # w2 is the stationary operand and hT the moving one, so the
                # stream covers exactly the ragged token count — no padded
                # columns and no partial-partition tiles.
                cs, c0 = c_chunks[ci], c_starts[ci]
                hs = hs_tiles.pop(ci)
                for ht in range(HK):
                    ps2 = ps2pool.tile([P, CHUNK], F32, tag="ps2")
                    for it in range(IT):
                        nc.tensor.matmul(
                            ps2[:, :cs],
                            w2s[:, it, ht * P : (ht + 1) * P],
                            hs[:, it, :cs],
                            start=(it == 0),
                            stop=(it == IT - 1),
                        )
                    ys = ypool.tile([P, CHUNK], F32, tag="ys")
                    nc.vector.tensor_mul(
                        out=ys[:, :cs], in0=ps2[:, :cs], in1=gb[:, c0 : c0 + cs]
                    )
                    nc.sync.dma_start(yT_v[ht][:, c0 : c0 + cs], ys[:, :cs])

            # software pipeline: run stage 1 a chunk ahead so the PE has
            # stage-1 work for chunk i+1 while w2 is still streaming in
            stage1(0)
            for ci in range(1, len(c_chunks)):
                stage1(ci)
                stage2(ci - 1)
            stage2(len(c_chunks) - 1)
    nc.compile()
    global LAST_NC
    LAST_NC = nc
    return nc


def route(router_logits):
    """Host-side router: softmax -> top-2 -> renormalize.

    Returns (top2_idx [T,2] int64, top2_gate [T,2] float32)."""
    logits = np.asarray(router_logits, dtype=np.float32)
    m = logits.max(axis=-1, keepdims=True)
    ex = np.exp(logits - m)
    probs = ex / ex.sum(axis=-1, keepdims=True)
    order = np.argsort(-probs, axis=-1, kind="stable")[:, :TOPK]
    rows = np.arange(logits.shape[0])[:, None]
    topk_p = probs[rows, order]
    topk_p = topk_p / topk_p.sum(axis=-1, keepdims=True)
    return order, topk_p.astype(np.float32)


def kernel(x, router_logits, w1, w2):
    x = np.ascontiguousarray(np.asarray(x, dtype=np.float32))
    w1 = np.asarray(w1, dtype=np.float32)
    w2 = np.asarray(w2, dtype=np.float32)
    t = x.shape[0]

    top2_idx, top2_gate = route(router_logits)

    expert_tokens = []
    expert_gates = []
    for e in range(E):
        sel = np.nonzero(top2_idx == e)
        expert_tokens.append(sel[0])
        expert_gates.append(top2_gate[sel[0], sel[1]])
    counts = [len(ix) for ix in expert_tokens]
    # fp32r matmuls require even free/partition sizes (2-element PSUM
    # interleave), so round the computed token count up to even
    count = max(2, max(counts) + max(counts) % 2)
    C = -(-count // P) * P  # buffer capacity (128-aligned)

    nc = build_moe_expert_kernel(count)
    kernel_cs0 = _count_chunks(count)[0]

    in_maps = []
    for e in range(E):
        cnt = counts[e]
        xT_e = np.zeros((H, C), dtype=np.float32)
        xT_e[:, :cnt] = x[expert_tokens[e]].T
        g = np.zeros((1, C), dtype=np.float32)
        g[0, :cnt] = expert_gates[e]
        w1T_e = np.ascontiguousarray(w1[e].T)
        cs0 = kernel_cs0
        in_maps.append(
            {
                "xT": xT_e,
                "w1T": w1T_e,
                "w2T": np.ascontiguousarray(w2[e].T),
                "gates": g,
                "prelude": np.ascontiguousarray(
                    np.concatenate([w1T_e[:P, :P], xT_e[:P, :cs0]], axis=1)
                ),
            }
        )

    res = run_bass_kernel_spmd(nc, in_maps, core_ids=list(range(N_CORES)))
    if not all(np.isfinite(r["yT"]).all() for r in res.results):
        # one retry in case of a transient device fault
        res = run_bass_kernel_spmd(nc, in_maps, core_ids=list(range(N_CORES)))

    out = np.zeros((t, H), dtype=np.float32)
    for e in range(E):
        cnt = counts[e]
        out[expert_tokens[e]] += res.results[e]["yT"][:, :cnt].T
    return out



# revision 2
# speedup vs baseline: 1.2420x; 1.2420x over previous
"""MoE (top-2 of 8 experts) Trainium2 kernel — fp8 DoubleRow version.

Strategy: expert-parallel across the 8 NeuronCores (host routes tokens,
core e computes expert e's MLP over its gathered tokens). The matmuls run
in fp8(e4m3) DoubleRow mode — one DR instruction contracts TWO 128-row
k-tiles in 0.5 cycles per output column (4x the fp32r rate) — with a
hi/lo split-correction that keeps the end-to-end relative error ~2e-3:

  operand a is stored as a_hi = e4m3(a) and a_lo = e4m3(a - a_hi); the
  product a·w is assembled from three rank-K products
      a_hi·w_hi + a_hi·w_lo + a_lo·w_hi       (a_lo·w_lo ~ 2^-8, dropped)
  The DR pair slots compute two rank-128 products per instruction:
    - "plain"  pairs two k-tiles of (a_hi, w_hi): the main term,
    - "paired" puts (w_hi, w_lo) against (a_lo, a_hi) of ONE k-tile: both
      correction terms in one instruction.
  Stage 1 (contraction H=1024, 8 k-tiles): 4 plain + 8 paired = 6 cyc/col
  Stage 2 (contraction I=1408, 11 k-tiles): 6 plain (one zero-padded) +
      11 paired = 8.5 cyc/col
  vs fp32r's 8 and 11 cyc/col — a 1.31x PE-time reduction, and the fp8
  operands halve the DMA bytes.

Scaling: w1 is host-scaled by SW1=32 (so its lo-part stays in e4m3 normal
range), making psum1 = 32·z. Sigmoid reads psum with scale 1/32; the DVE
multiply gives hv = 32·silu(z) (absmax ~212 < e4m3 max 240), which is
split hi/lo for stage 2. w2 is scaled by SW2=32 and the host pre-divides
the gates by SW1·SW2 so the stage-2 gate-multiply absorbs all scales.

Per-core device pipeline (count = max tokens routed to one expert):
  stage 1, chunk-outer: psum[it] group (full 2KB bank, two 256-col DR
    half-sweeps) -> ACT sigmoid -> DVE mul (hv) -> ACT copy-cast (h_hi)
    -> GpSimd sub (h_lo), writing h into hlh [p, slot(lo,hi,zero), it, C]
  stage 2: psum[ht] group -> DVE gate-mul -> DMA out yT [H, C] fp32.
The host transposes and scatter-adds the two expert contributions.
"""

import numpy as np
import ml_dtypes

import concourse.mybir as mybir
from concourse import bacc
from concourse.tile import TileContext
from concourse.bass_utils import run_bass_kernel_spmd

T, H, I, E = 4096, 1024, 1408, 8
TOPK = 2
P = 128
HK = H // P  # 8
IT = I // P  # 11
N_CORES = 8
F32 = mybir.dt.float32
F8 = mybir.dt.float8e4
E4 = ml_dtypes.float8_e4m3
AF = mybir.ActivationFunctionType
DR = mybir.MatmulPerfMode.DoubleRow
SW1 = 32.0
SW2 = 32.0

# most recently built device program (for test harnesses / cost-model timing)
LAST_NC = None


def _chunks(count):
    """Column chunks: a small first chunk so the first psum group is armed
    after ~0.8 MB of DMA, then 512-wide (one full PSUM bank). All even."""
    out = []
    rem = count
    first = min(256, rem)
    out.append(first)
    rem -= first
    while rem > 0:
        c = min(512, rem)
        out.append(c)
        rem -= c
    return out


def _halves(cs):
    """Split a chunk into DR-sized half-sweeps (moving free dim 2*cols must
    stay <= 512, so <= 256 output columns per DR matmul); halves stay even."""
    if cs <= 256:
        return [(0, cs)]
    h0 = (cs // 2 + 1) // 2 * 2
    return [(0, h0), (h0, cs - h0)]


def build_moe_expert_kernel(count):
    """One-expert MLP over `count` gathered tokens (even)."""
    C = count
    assert count % 2 == 0
    nc = bacc.Bacc("TRN2", target_bir_lowering=False, debug=False, num_devices=N_CORES)

    xlh_d = nc.dram_tensor("xlh", [P, 2 * HK * C], F8, kind="ExternalInput").ap()
    w1_d = nc.dram_tensor("whl1", [P, IT * 2 * HK * P], F8, kind="ExternalInput").ap()
    w2_d = nc.dram_tensor("whl2", [P, HK * IT * 2 * P], F8, kind="ExternalInput").ap()
    g_d = nc.dram_tensor("gates", [1, C], F32, kind="ExternalInput").ap()
    y_d = nc.dram_tensor("yT", [H, C], F32, kind="ExternalOutput").ap()

    # logical views (slot order: w (hi, lo); x and h (lo, hi[, zero]))
    xlh_v = xlh_d.rearrange("p (s k c) -> p s k c", s=2, k=HK)
    w1_v = w1_d.rearrange("p (i s k j) -> p i s k j", i=IT, s=2, k=HK)
    w2_v = w2_d.rearrange("p (h i s j) -> p h i s j", h=HK, i=IT, s=2)
    y_v = y_d.rearrange("(h p) c -> h p c", p=P)  # [HK, 128, C]

    cks = _chunks(count)
    c_starts = [sum(cks[:j]) for j in range(len(cks))]

    with TileContext(nc) as tc:
        with (
            tc.tile_pool(name="w", bufs=1) as wpool,
            tc.tile_pool(name="hv", bufs=3) as hvpool,
            tc.tile_pool(name="y", bufs=3) as ypool,
            tc.tile_pool(name="ps1", bufs=4, space="PSUM") as ps1p,
            tc.tile_pool(name="ps2", bufs=4, space="PSUM") as ps2p,
        ):
            wt1 = wpool.tile([P, IT, 2, HK, P], F8)
            wt2 = wpool.tile([P, HK, IT, 2, P], F8)
            xt = wpool.tile([P, 2, HK, C], F8)
            hlh = wpool.tile([P, 3, IT, C], F8)
            gb = wpool.tile([P, C], F32)

            # DMA issue order = consumption order: w1 it0, x chunk 0 (arms
            # the first psum group), the rest of w1 (paced by the chunk-0
            # it-sweep), remaining x chunks, gates, then w2 per ht.
            nc.sync.dma_start(wt1[:, 0], w1_v[:, 0])
            nc.sync.dma_start(
                xt[:, :, :, : cks[0]], xlh_v[:, :, :, : cks[0]]
            )
            for it in range(1, IT):
                nc.sync.dma_start(wt1[:, it], w1_v[:, it])
            for c0, cs in zip(c_starts[1:], cks[1:]):
                nc.sync.dma_start(
                    xt[:, :, :, c0 : c0 + cs], xlh_v[:, :, :, c0 : c0 + cs]
                )
            nc.sync.dma_start(gb[:], g_d[0].partition_broadcast(P))
            for ht in range(HK):
                nc.sync.dma_start(wt2[:, ht], w2_v[:, ht])

            # the only zero-slot region stage 2 ever reads (it10 plain term)
            nc.vector.memset(hlh[:, 2, IT - 1, :], 0.0)

            def s1_group(it, c0, cs):
                ps = ps1p.tile([P, 512], F32, tag="ps1")
                for h0, hcs in _halves(cs):
                    a, b = c0 + h0, c0 + h0 + hcs
                    for hkp in range(0, HK, 2):  # plain: x_hi @ w1_hi
                        nc.tensor.matmul(
                            ps[:, h0 : h0 + hcs],
                            wt1[:, it, 0, hkp : hkp + 2, :],
                            xt[:, 1, hkp : hkp + 2, a:b],
                            start=(h0 == 0 and hkp == 0),
                            stop=False,
                            perf_mode=DR,
                        )
                    for hk in range(HK):  # paired: w_hi*x_lo + w_lo*x_hi
                        nc.tensor.matmul(
                            ps[:, h0 : h0 + hcs],
                            wt1[:, it, :, hk, :],
                            xt[:, :, hk, a:b],
                            start=False,
                            stop=(h0 + hcs == cs and hk == HK - 1),
                            perf_mode=DR,
                        )
                # evacuate: hv = psum * sigmoid(psum/SW1) = SW1*silu(z),
                # then split h into e4m3 hi/lo for stage 2
                sg = hvpool.tile([P, 512], F32, tag="sg")
                nc.scalar.activation(
                    sg[:, :cs], ps[:, :cs], AF.Sigmoid, scale=1.0 / SW1
                )
                hv = hvpool.tile([P, 512], F32, tag="hv")
                nc.vector.tensor_mul(out=hv[:, :cs], in0=ps[:, :cs], in1=sg[:, :cs])
                nc.scalar.activation(hlh[:, 1, it, c0 : c0 + cs], hv[:, :cs], AF.Copy)
                nc.gpsimd.tensor_sub(
                    hlh[:, 0, it, c0 : c0 + cs],
                    hv[:, :cs],
                    hlh[:, 1, it, c0 : c0 + cs],
                )

            def s2_group(ht, c0, cs):
                ps = ps2p.tile([P, 512], F32, tag="ps2")
                for h0, hcs in _halves(cs):
                    a, b = c0 + h0, c0 + h0 + hcs
                    for itp in range(0, IT - 1, 2):  # plain: h_hi @ w2_hi
                        nc.tensor.matmul(
                            ps[:, h0 : h0 + hcs],
                            wt2[:, ht, itp : itp + 2, 0, :],
                            hlh[:, 1, itp : itp + 2, a:b],
                            start=(h0 == 0 and itp == 0),
                            stop=False,
                            perf_mode=DR,
                        )
                    # it10 plain, zero-padded second slot
                    nc.tensor.matmul(
                        ps[:, h0 : h0 + hcs],
                        wt2[:, ht, IT - 1, :, :],
                        hlh[:, 1:3, IT - 1, a:b],
                        start=False,
                        stop=False,
                        perf_mode=DR,
                    )
                    for it in range(IT):  # paired: w2_hi*h_lo + w2_lo*h_hi
                        nc.tensor.matmul(
                            ps[:, h0 : h0 + hcs],
                            wt2[:, ht, it, :, :],
                            hlh[:, 0:2, it, a:b],
                            start=False,
                            stop=(h0 + hcs == cs and it == IT - 1),
                            perf_mode=DR,
                        )
                ys = ypool.tile([P, 512], F32, tag="ys")
                nc.vector.tensor_mul(
                    out=ys[:, :cs], in0=ps[:, :cs], in1=gb[:, c0 : c0 + cs]
                )
                nc.sync.dma_start(y_v[ht][:, c0 : c0 + cs], ys[:, :cs])

            for c0, cs in zip(c_starts, cks):  # stage 1, chunk-outer
                for it in range(IT):
                    s1_group(it, c0, cs)
            for c0, cs in zip(c_starts, cks):  # stage 2
                for ht in range(HK):
                    s2_group(ht, c0, cs)

    nc.compile()
    global LAST_NC
    LAST_NC = nc
    return nc


def route(router_logits):
    """Host-side router: softmax -> top-2 -> renormalize."""
    logits = np.asarray(router_logits, dtype=np.float32)
    m = logits.max(axis=-1, keepdims=True)
    ex = np.exp(logits - m)
    probs = ex / ex.sum(axis=-1, keepdims=True)
    order = np.argsort(-probs, axis=-1, kind="stable")[:, :TOPK]
    rows = np.arange(logits.shape[0])[:, None]
    topk_p = probs[rows, order]
    topk_p = topk_p / topk_p.sum(axis=-1, keepdims=True)
    return order, topk_p.astype(np.float32)


def _q8(a):
    return np.asarray(a, dtype=np.float32).astype(E4)


def kernel(x, router_logits, w1, w2):
    x = np.ascontiguousarray(np.asarray(x, dtype=np.float32))
    w1 = np.asarray(w1, dtype=np.float32)
    w2 = np.asarray(w2, dtype=np.float32)
    t = x.shape[0]

    top2_idx, top2_gate = route(router_logits)

    expert_tokens = []
    expert_gates = []
    for e in range(E):
        sel = np.nonzero(top2_idx == e)
        expert_tokens.append(sel[0])
        expert_gates.append(top2_gate[sel[0], sel[1]])
    counts = [len(ix) for ix in expert_tokens]
    count = max(2, max(counts) + max(counts) % 2)

    nc = build_moe_expert_kernel(count)

    in_maps = []
    for e in range(E):
        cnt = counts[e]
        xe = x[expert_tokens[e]]  # [cnt, H]
        x_hi = _q8(xe)
        x_lo = _q8(xe - x_hi.astype(np.float32))
        xlh = np.zeros((P, 2, HK, count), dtype=E4)
        xlh[:, 0, :, :cnt] = x_lo.reshape(cnt, HK, P).transpose(2, 1, 0)
        xlh[:, 1, :, :cnt] = x_hi.reshape(cnt, HK, P).transpose(2, 1, 0)

        W1 = SW1 * w1[e]  # [I, H]
        W1_hi = _q8(W1)
        W1_lo = _q8(W1 - W1_hi.astype(np.float32))
        # whl1[p, it, slot, hk, j] = W1_s[it*128+j, hk*128+p]
        w1hi_t = W1_hi.reshape(IT, P, HK, P).transpose(3, 0, 2, 1)
        w1lo_t = W1_lo.reshape(IT, P, HK, P).transpose(3, 0, 2, 1)
        whl1 = np.stack([w1hi_t, w1lo_t], axis=2)  # [p, it, 2, hk, j]

        W2 = SW2 * w2[e]  # [H, I]
        W2_hi = _q8(W2)
        W2_lo = _q8(W2 - W2_hi.astype(np.float32))
        # whl2[p, ht, it, slot, j] = W2_s[ht*128+j, it*128+p]
        w2hi_t = W2_hi.reshape(HK, P, IT, P).transpose(3, 0, 2, 1)
        w2lo_t = W2_lo.reshape(HK, P, IT, P).transpose(3, 0, 2, 1)
        whl2 = np.stack([w2hi_t, w2lo_t], axis=3)  # [p, ht, it, 2, j]

        g = np.zeros((1, count), dtype=np.float32)
        g[0, :cnt] = expert_gates[e] / (SW1 * SW2)

        in_maps.append(
            {
                "xlh": np.ascontiguousarray(xlh).reshape(P, -1),
                "whl1": np.ascontiguousarray(whl1).reshape(P, -1),
                "whl2": np.ascontiguousarray(whl2).reshape(P, -1),
                "gates": g,
            }
        )

    res = run_bass_kernel_spmd(nc, in_maps, core_ids=list(range(N_CORES)))
    if not all(np.isfinite(r["yT"]).all() for r in res.results):
        # one retry in case of a transient device fault
        res = run_bass_kernel_spmd(nc, in_maps, core_ids=list(range(N_CORES)))

    out = np.zeros((t, H), dtype=np.float32)
    for e in range(E):
        cnt = counts[e]
        out[expert_tokens[e]] += res.results[e]["yT"][:, :cnt].T
    return out
